# revision 1
# baseline (speedup 1.0000x reference)
"""Trainium2 Bass kernel for nn_Decoder (sparse band attention decoder).

Computation per batch n (E=512, T=2048, H=8 heads, d=64, band |q-k|<=64):
    x = mask[n].T; qkv = x @ w_in.T + b_in; q,k,v = split(qkv)
    o = band_softmax(q k^T/sqrt(d)) @ v; o = o @ w_out.T + b_out
    y = relu(o).T * encodings[n]; out = w_ct.T @ y + b_ct

Sharding: pure data-parallel over batch N=8 across 8 NeuronCores (one batch
per core, no collectives). Per-core dataflow is fully "transposed" ([E, T])
so nothing ever needs a DMA transpose; the 129-wide attention band is
evaluated on 256-wide key windows per 128-row query tile:
  - Q^T, K^T computed as [e_out, t] tiles (lhsT = w_in^T, rhs = mask[n])
  - V computed as [t, e_out] tiles   (lhsT = mask[n], rhs = w_in^T)
  - band mask added in-PSUM via an identity-matmul accumulate
  - softmax without max-subtraction (scores are O(10)); row sums via DVE
    tensor_scalar accumulate; normalization in the [q, k] layout
  - normalized attention transposed 128x128 on the PE and fed to the
    o-matmul as the moving operand (o^T[d, q] accumulates in PSUM)
  - out_proj / conv-transpose (k=1 == linear) are plain matmuls in [E, T]

The out_proj/conv tail is interleaved into the attention loop: t-chunk t4
is projected as soon as query tiles 4*t4..4*t4+3 are done, sharing PSUM
banks with the attention pools, and output DMA streams out per chunk.

Engine discipline: this walrus build rejects any instruction carrying more
than one semaphore wait, so _split_excess_waits post-processes the BIR and
moves excess waits onto same-engine no-op carrier instructions.
"""

import numpy as np
import ml_dtypes

import concourse.bass as bass
import concourse.mybir as mybir
from concourse import tile
from concourse.bass_utils import run_bass_kernel_spmd

BF16NP = ml_dtypes.bfloat16
N, E, T, H, D = 8, 512, 2048, 8, 64
LOCAL = 64
WIN = 256
NEG = -50.0
NQT = T // 128  # 16 query tiles

F32 = mybir.dt.float32
BF = mybir.dt.bfloat16


def _window_start(qt: int) -> int:
    return min(max(128 * qt - 64, 0), T - WIN)


def _chunk_rows(qt: int):
    ws = _window_start(qt)
    return (ws, ws + 128)


def _v_row_starts():
    rows = set()
    for qt in range(NQT):
        rows.update(_chunk_rows(qt))
    return sorted(rows)


def _split_excess_waits(nc: bass.Bass, limit: int = 1) -> None:
    """This walrus build rejects instructions carrying more than one sync
    wait ("Too many sync wait commands"), so redistribute excess waits onto
    same-engine no-op carrier instructions inserted just before the owner.
    """
    eng_map = {
        mybir.EngineType.SP: nc.sync,
        mybir.EngineType.Pool: nc.gpsimd,
        mybir.EngineType.PE: nc.tensor,
        mybir.EngineType.DVE: nc.vector,
        mybir.EngineType.Activation: nc.scalar,
    }
    for f in nc.m.functions:
        for bb in f.blocks:
            insts = bb.instructions
            offenders = []
            for idx, inst in enumerate(insts):
                si = getattr(inst, "sync_info", None)
                if si is not None and si.on_wait and len(si.on_wait) > limit:
                    offenders.append((idx, inst))
            if not offenders:
                continue
            for idx, inst in reversed(offenders):
                si = inst.sync_info
                waits = list(si.on_wait)
                keep, excess = waits[-limit:], waits[:-limit]
                eng = eng_map[inst.engine]
                carriers = []
                for j in range(0, len(excess), limit):
                    nop = eng.nop().ins
                    # eng.nop() appended to the builder's current bb; reclaim it
                    cur = nc.cur_bb.bb.instructions
                    assert cur and cur[-1] is nop
                    cur.pop()
                    nop.sync_info = mybir.SyncInfo(
                        on_wait=excess[j : j + limit], on_update=[]
                    )
                    carriers.append(nop)
                inst.sync_info = mybir.SyncInfo(
                    on_wait=keep, on_update=list(si.on_update)
                )
                for c in reversed(carriers):
                    insts.insert(idx, c)


def _build_nc(with_vbias: bool = True) -> bass.Bass:
    nc = bass.Bass()

    xm_d = nc.declare_dram_parameter("xm", [E, T], BF, False)
    enc_d = nc.declare_dram_parameter("enc", [E, T], BF, False)
    w_inT_d = nc.declare_dram_parameter("w_inT", [E, 3 * E], BF, False)
    b_in_d = nc.declare_dram_parameter("b_in", [3 * E], F32, False)
    bv_d = nc.declare_dram_parameter("bv_row", [1, E], F32, False)
    ones_d = nc.declare_dram_parameter("ones_col", [1, 128], F32, False)
    w_outT_d = nc.declare_dram_parameter("w_outT", [E, E], BF, False)
    b_out_d = nc.declare_dram_parameter("b_out", [E], F32, False)
    w_ct_d = nc.declare_dram_parameter("w_ct", [E, E], BF, False)
    b_ct_d = nc.declare_dram_parameter("b_ct", [E], F32, False)
    bmask_d = nc.declare_dram_parameter("bmask", [128, 3 * WIN], BF, False)
    ident_d = nc.declare_dram_parameter("ident", [128, 128], BF, False)
    out_d = nc.declare_dram_parameter("out", [E, T], F32, True)

    vrows = _v_row_starts()

    with tile.TileContext(nc) as tc, tc.tile_pool(name="persist", bufs=1) as pp:
        with (
            tc.tile_pool(name="phaseA", bufs=1) as pa,
            tc.tile_pool(name="psA", bufs=1, space="PSUM") as psA,
        ):
            # ---- persistent small constants (DMA -> stage -> gpsimd copy) ----
            ident_sb = pp.tile([128, 128], BF, tag="ident", name="ident")
            bmask_sb = pp.tile([128, 3 * WIN], BF, tag="bmask", name="bmask")
            ones_sb = pp.tile([1, 128], F32, tag="ones", name="ones")
            bv_sb = pp.tile([1, E], F32, tag="bv", name="bv")
            bqk_sb = pp.tile([128, 8], F32, tag="bqk", name="bqk")
            bout_sb = pp.tile([128, 4], F32, tag="bout", name="bout")
            bct_sb = pp.tile([128, 4], F32, tag="bct", name="bct")


            # ---- phase-A inputs, staged per 512-column chunk ----
            xm_sb = [
                pa.tile([128, T], BF, tag=f"xm{k}", name=f"xm{k}") for k in range(4)
            ]
            wi_sb = [
                pa.tile([128, 3 * E], BF, tag=f"wi{k}", name=f"wi{k}")
                for k in range(4)
            ]
            for k in range(4):
                nc.gpsimd.dma_start(
                    wi_sb[k][:, 1024:1536], w_inT_d[k * 128 : (k + 1) * 128, 1024:1536]
                )
            for k in range(4):
                for c in range(4):
                    eng = nc.sync if (k * 4 + c) % 2 == 0 else nc.scalar
                    eng.dma_start(
                        xm_sb[k][:, c * 512 : (c + 1) * 512],
                        xm_d[k * 128 : (k + 1) * 128, c * 512 : (c + 1) * 512],
                    )
            for k in range(4):
                nc.gpsimd.dma_start(
                    wi_sb[k][:, 0:1024], w_inT_d[k * 128 : (k + 1) * 128, 0:1024]
                )
            nc.gpsimd.dma_start(ones_sb[:], ones_d[:])
            nc.gpsimd.dma_start(bv_sb[:], bv_d[:])
            nc.gpsimd.dma_start(ident_sb[:], ident_d[:])
            nc.gpsimd.dma_start(bmask_sb[:], bmask_d[:])
            nc.gpsimd.dma_start(
                bqk_sb[:], b_in_d[0:1024].rearrange("(e p) -> p e", p=128)
            )
            nc.gpsimd.dma_start(
                bout_sb[:], b_out_d[:].rearrange("(e p) -> p e", p=128)
            )
            nc.gpsimd.dma_start(
                bct_sb[:], b_ct_d[:].rearrange("(e p) -> p e", p=128)
            )

            # ---- persistent activations ----
            qkt = [
                pp.tile([128, T], BF, tag=f"qkt{i}", name=f"qkt{i}") for i in range(8)
            ]
            vt = {
                r: pp.tile([128, E], BF, tag=f"v{r}", name=f"v{r}") for r in vrows
            }
            # o^T as one [128, 4*T] tensor: e_in row k*128+p lives at
            # partition p, column block k*T (see o eviction below)
            oT_sb = pp.tile([128, 4 * T], BF, tag="oT", name="oT")

            # ---- phase A: V tiles first (ACT evictions precede Q/K's) ----
            for r in vrows:
                ps = psA.tile([128, 512], F32, tag="psv", name="psv", bufs=2)
                for k in range(4):
                    nc.tensor.matmul(
                        ps[:],
                        xm_sb[k][:, r : r + 128],
                        wi_sb[k][:, 1024:1536],
                        start=(k == 0),
                        stop=(k == 3 and not with_vbias),
                    )
                if with_vbias:
                    nc.tensor.matmul(
                        ps[:], ones_sb[:], bv_sb[:], start=False, stop=True
                    )
                if (r // 64) % 2 == 0:
                    nc.scalar.copy(vt[r][:], ps[:])
                else:
                    nc.vector.tensor_copy(vt[r][:], ps[:])

            # ---- phase A: Q^T, K^T ----
            for eo in range(8):
                for t4 in range(4):
                    ps = psA.tile(
                        [128, 512], F32, tag=f"psa{t4}", name=f"psa{t4}"
                    )
                    for k in range(4):
                        nc.tensor.matmul(
                            ps[:],
                            wi_sb[k][:, eo * 128 : (eo + 1) * 128],
                            xm_sb[k][:, t4 * 512 : (t4 + 1) * 512],
                            start=(k == 0),
                            stop=(k == 3),
                        )
                    if t4 % 2 == 0:
                        nc.scalar.activation(
                            qkt[eo][:, t4 * 512 : (t4 + 1) * 512],
                            ps[:],
                            mybir.ActivationFunctionType.Identity,
                            bias=bqk_sb[:, eo : eo + 1],
                        )
                    else:
                        nc.vector.tensor_scalar(
                            qkt[eo][:, t4 * 512 : (t4 + 1) * 512],
                            ps[:],
                            bqk_sb[:, eo : eo + 1],
                            None,
                            mybir.AluOpType.add,
                        )

        # ---- attention with interleaved out_proj/conv tail ----
        wo_sb = [pp.tile([128, E], BF, tag=f"wo{k}", name=f"wo{k}") for k in range(4)]
        wc_sb = [pp.tile([128, E], BF, tag=f"wc{k}", name=f"wc{k}") for k in range(4)]
        enc_sb = [pp.tile([128, T], BF, tag=f"enc{k}", name=f"enc{k}") for k in range(4)]
        y_sb = [pp.tile([128, T], BF, tag=f"y{k}", name=f"y{k}") for k in range(4)]
        out_sb = [pp.tile([128, T], F32, tag=f"os{k}", name=f"os{k}") for k in range(4)]
        for k in range(4):
            nc.gpsimd.dma_start(wo_sb[k][:], w_outT_d[k * 128 : (k + 1) * 128, :])
            nc.gpsimd.dma_start(wc_sb[k][:], w_ct_d[k * 128 : (k + 1) * 128, :])
            nc.gpsimd.dma_start(enc_sb[k][:], enc_d[k * 128 : (k + 1) * 128, :])
        with (
            tc.tile_pool(name="psS", bufs=2, space="PSUM") as psS,
            tc.tile_pool(name="psT", bufs=2, space="PSUM") as psT,
            tc.tile_pool(name="psO", bufs=2, space="PSUM") as psO,
            tc.tile_pool(name="attn", bufs=8) as ap,
            tc.tile_pool(name="small", bufs=2) as sp,
            tc.tile_pool(name="tmp3", bufs=3) as tp3,
        ):
            for qt in range(NQT):
                ws = _window_start(qt)
                m = 0 if qt == 0 else (2 if qt == NQT - 1 else 1)
                l_t = sp.tile([128, 8], F32, tag="l", name="l")
                r_t = sp.tile([128, 8], F32, tag="r", name="r")
                aes = []
                for hp in range(4):
                    ps_s = psS.tile([128, 512], F32, tag="s", name="s")
                    for hh in range(2):
                        h = 2 * hp + hh
                        nc.tensor.matmul(
                            ps_s[:, hh * WIN : (hh + 1) * WIN],
                            qkt[h // 2][
                                (h % 2) * 64 : (h % 2) * 64 + 64,
                                qt * 128 : (qt + 1) * 128,
                            ],
                            qkt[4 + h // 2][
                                (h % 2) * 64 : (h % 2) * 64 + 64, ws : ws + WIN
                            ],
                            start=(hh == 0),
                            stop=False,
                        )
                        nc.tensor.matmul(
                            ps_s[:, hh * WIN : (hh + 1) * WIN],
                            ident_sb[:],
                            bmask_sb[:, m * WIN : (m + 1) * WIN],
                            start=False,
                            stop=(hh == 1),
                        )
                    ae = ap.tile([128, 512], BF, tag="ae", name="ae")
                    nc.scalar.activation(
                        ae[:], ps_s[:], mybir.ActivationFunctionType.Exp
                    )
                    for hh in range(2):
                        h = 2 * hp + hh
                        scr = ap.tile([128, WIN], BF, tag="scr", name="scr")
                        nc.vector.tensor_scalar(
                            scr[:],
                            ae[:, hh * WIN : (hh + 1) * WIN],
                            1.0,
                            None,
                            mybir.AluOpType.mult,
                            op1=mybir.AluOpType.add,
                            accum_out=l_t[:, h : h + 1],
                        )
                    aes.append(ae)
                    nc.vector.reciprocal(
                        r_t[:, 2 * hp : 2 * hp + 2], l_t[:, 2 * hp : 2 * hp + 2]
                    )

                ans = []
                for hp in range(4):
                    an = ap.tile([128, 512], BF, tag="an", name="an")
                    for hh in range(2):
                        h = 2 * hp + hh
                        nc.vector.tensor_scalar(
                            an[:, hh * WIN : (hh + 1) * WIN],
                            aes[hp][:, hh * WIN : (hh + 1) * WIN],
                            r_t[:, h : h + 1],
                            None,
                            mybir.AluOpType.mult,
                        )
                    ans.append(an)

                # transpose the 16 [128,128] chunks; chunk idx = h*2 + c
                aT_ps = [
                    psT.tile([128, 1024], BF, tag="aTp", name="aTp") for _ in range(2)
                ]
                for h in range(H):
                    for c in range(2):
                        idx = h * 2 + c
                        nc.tensor.transpose(
                            aT_ps[idx // 8][
                                :, (idx % 8) * 128 : (idx % 8) * 128 + 128
                            ],
                            ans[h // 2][
                                :,
                                (h % 2) * WIN + c * 128 : (h % 2) * WIN + c * 128 + 128,
                            ],
                            ident_sb[:],
                        )
                aT_sb = [
                    ap.tile([128, 1024], BF, tag=f"aTs{i}", name=f"aTs{i}")
                    for i in range(2)
                ]
                nc.vector.tensor_copy(aT_sb[0][:], aT_ps[0][:])
                nc.scalar.copy(aT_sb[1][:], aT_ps[1][:])

                # o^T: head h -> partitions (h%2)*64, cols (h//2)*128
                o_ps = psO.tile([128, 512], F32, tag="o", name="o")
                rows = _chunk_rows(qt)
                for h in range(H):
                    for c in range(2):
                        idx = h * 2 + c
                        nc.tensor.matmul(
                            o_ps[
                                (h % 2) * 64 : (h % 2) * 64 + 64,
                                (h // 2) * 128 : (h // 2) * 128 + 128,
                            ],
                            vt[rows[c]][:, h * 64 : (h + 1) * 64],
                            aT_sb[idx // 8][
                                :, (idx % 8) * 128 : (idx % 8) * 128 + 128
                            ],
                            start=(c == 0),
                            stop=(c == 1),
                        )
                dst = oT_sb[:].rearrange("p (k t) -> p k t", k=4)[
                    :, :, qt * 128 : (qt + 1) * 128
                ]
                src = o_ps[:].rearrange("p (k t) -> p k t", k=4)
                if qt % 2 == 0:
                    nc.vector.tensor_copy(dst, src)
                else:
                    nc.scalar.copy(dst, src)

                tails = []
                if qt >= 5 and qt % 4 == 1:
                    t4 = qt // 4 - 1
                    tails = [(t4 * 512, 512)]
                if qt == NQT - 2:
                    tails = tails + [(1536, 256)]
                if qt == NQT - 1:
                    tails = tails + [(1792, 256)]
                for c0, cw in tails:
                    for eo in range(4):
                        ps = psO.tile([128, 512], F32, tag="tl", name="tl")
                        for k in range(4):
                            nc.tensor.matmul(
                                ps[:, 0:cw],
                                wo_sb[k][:, eo * 128 : (eo + 1) * 128],
                                oT_sb[:, k * T + c0 : k * T + c0 + cw],
                                start=(k == 0),
                                stop=(k == 3),
                            )
                        y1 = tp3.tile([128, 512], BF, tag="y1", name="y1")
                        nc.scalar.activation(
                            y1[:, 0:cw],
                            ps[:, 0:cw],
                            mybir.ActivationFunctionType.Relu,
                            bias=bout_sb[:, eo : eo + 1],
                        )
                        nc.vector.tensor_tensor(
                            y_sb[eo][:, c0 : c0 + cw],
                            y1[:, 0:cw],
                            enc_sb[eo][:, c0 : c0 + cw],
                            mybir.AluOpType.mult,
                        )
                    for eo in range(4):
                        ps = psO.tile([128, 512], F32, tag="tl", name="tl")
                        for k in range(4):
                            nc.tensor.matmul(
                                ps[:, 0:cw],
                                wc_sb[k][:, eo * 128 : (eo + 1) * 128],
                                y_sb[k][:, c0 : c0 + cw],
                                start=(k == 0),
                                stop=(k == 3),
                            )
                        nc.vector.tensor_scalar(
                            out_sb[eo][:, c0 : c0 + cw],
                            ps[:, 0:cw],
                            bct_sb[:, eo : eo + 1],
                            None,
                            mybir.AluOpType.add,
                        )
                        nc.sync.dma_start(
                            out_d[eo * 128 : (eo + 1) * 128, c0 : c0 + cw],
                            out_sb[eo][:, c0 : c0 + cw],
                        )

    _split_excess_waits(nc)
    return nc


def _band_masks() -> np.ndarray:
    qr = np.arange(128)[:, None]
    col = np.arange(WIN)[None, :]
    m0 = np.where(np.abs(qr - col) <= LOCAL, 0.0, NEG)
    m1 = np.where(np.abs(qr + 64 - col) <= LOCAL, 0.0, NEG)
    m2 = np.where(np.abs(128 + qr - col) <= LOCAL, 0.0, NEG)
    return np.concatenate([m0, m1, m2], axis=1).astype(BF16NP)


_NC_CACHE = None
_last_in_maps = None


def kernel(mask, encodings, w_in, b_in, w_out, b_out, w_ct, b_ct, local_size):
    global _NC_CACHE, _last_in_maps
    mask = np.asarray(mask, np.float32)
    encodings = np.asarray(encodings, np.float32)
    w_in = np.asarray(w_in, np.float32)
    b_in = np.asarray(b_in, np.float32)
    w_out = np.asarray(w_out, np.float32)
    b_out = np.asarray(b_out, np.float32)
    w_ct = np.asarray(w_ct, np.float32)
    b_ct = np.asarray(b_ct, np.float32)
    assert int(local_size) == LOCAL and mask.shape == (N, E, T)

    scale = 1.0 / np.sqrt(E // H)
    w_inT = np.ascontiguousarray(w_in.T).astype(np.float32)
    w_inT[:, :E] *= scale
    b_in2 = b_in.copy()
    b_in2[:E] *= scale

    shared = {
        "w_inT": w_inT.astype(BF16NP),
        "b_in": b_in2,
        "bv_row": np.ascontiguousarray(b_in2[2 * E :][None, :]),
        "ones_col": np.ones((1, 128), np.float32),
        "w_outT": np.ascontiguousarray(w_out.T).astype(BF16NP),
        "b_out": b_out,
        "w_ct": w_ct.astype(BF16NP),
        "b_ct": b_ct,
        "bmask": _band_masks(),
        "ident": np.eye(128, dtype=BF16NP),
    }
    in_maps = []
    for n in range(N):
        m = dict(shared)
        m["xm"] = np.ascontiguousarray(mask[n]).astype(BF16NP)
        m["enc"] = encodings[n].astype(BF16NP)
        in_maps.append(m)
    _last_in_maps = in_maps

    if _NC_CACHE is None:
        _NC_CACHE = _build_nc(with_vbias=bool(np.any(b_in2[2 * E :])))
    res = run_bass_kernel_spmd(_NC_CACHE, in_maps, list(range(N)))
    out = np.stack([np.asarray(res.results[n]["out"]) for n in range(N)], axis=0)
    return out.astype(np.float32)


if __name__ == "__main__":
    rng = np.random.default_rng(0)
    inputs = {
        "mask": rng.standard_normal((N, E, T)).astype(np.float32),
        "encodings": rng.standard_normal((N, E, T)).astype(np.float32),
        "w_in": (rng.standard_normal((3 * E, E)) / np.sqrt(E)).astype(np.float32),
        "b_in": np.zeros(3 * E, np.float32),
        "w_out": (rng.standard_normal((E, E)) / np.sqrt(E)).astype(np.float32),
        "b_out": np.zeros(E, np.float32),
        "w_ct": (rng.standard_normal((E, E)) / np.sqrt(E)).astype(np.float32),
        "b_ct": np.zeros(E, np.float32),
        "local_size": LOCAL,
    }
    out = kernel(**inputs)
    print("kernel ran, out shape", out.shape, "absmax", np.abs(out).max())

